# revision 20
# baseline (speedup 1.0000x reference)
"""GCN block (4x GCNConv w/ symmetric norm + self-loops + ReLU) on 8 TRN2 NeuronCores.

Strategy (dst-sharding, v2):
  - Nodes are bin-packed (by degree incl. self-loop) into 128-slot "tiles";
    each core owns NT tiles. Edges (incl. explicit self-loop edges) are
    partitioned by the tile of their *destination* and packed into 128-edge
    chunks (C chunks per tile, uniform).
  - Symmetric normalization is factored out of the edge weights:
      stored_l = D^-1/2 x_l   (layer-0 input pre-scaled on host)
      agg_raw  = (A+I) stored_l          (0/1 selection matmuls)
      stored_{l+1} = relu(dinv^2 (.) (agg_raw @ W))   [per-dst-row scale]
      out (last)   = relu(dinv   (.) (agg_raw @ W))
    so the S matrices are pure 0/1 -> stored once in SBUF as fp8 for all
    layers, and the scales ride the scalar-engine activation for free.
  - Per layer, per core, per group of TG=4 tiles:
      one batched indirect DMA gathers all TG*C chunks' source rows (fp16)
      from the AllGather'd node-feature buffer; C matmuls per tile accumulate
      agg^T in PSUM (tokens stationary, S moving); agg^T feeds the weight
      matmul directly; activation applies scale+relu; rows stored fp16.
  - Layer-0 features are replicated to every core as an input (no first
    AllGather); layers 1..3 AllGather fp16 shards (3.4MB/rank).
"""

import math
import os
import sys

import numpy as np

sys.path.insert(0, "/opt/trn_rl_repo")

NCORES = 8
P = 128          # SBUF partitions == slots per tile == edge-chunk size
D = 128          # feature dim
TG = 4           # tiles per group (4*128 fp32 = one full PSUM bank)

_CACHE = {}


# ----------------------------------------------------------------------------
# Host-side preprocessing (indices / metadata only)
# ----------------------------------------------------------------------------

def _assign_tiles(deg, n_tiles):
    """Balance nodes into n_tiles bins by degree, capacity 128 nodes/bin.

    Returns (tile_of[n], slot_of[n]).
    """
    import heapq

    n_nodes = deg.shape[0]
    assert n_tiles * P >= n_nodes
    order = np.argsort(-deg, kind="stable")
    heap = [(0, t) for t in range(n_tiles)]
    heapq.heapify(heap)
    counts = np.zeros(n_tiles, np.int32)
    tile_of = np.empty(n_nodes, np.int32)
    slot_of = np.empty(n_nodes, np.int32)
    for n in order:
        load, t = heapq.heappop(heap)
        tile_of[n] = t
        slot_of[n] = counts[t]
        counts[t] += 1
        if counts[t] < P:
            heapq.heappush(heap, (load + int(deg[n]), t))
    return tile_of, slot_of


def _preprocess(edge_index, n_nodes, nt_per_core):
    """Build all per-core index/metadata arrays."""
    import ml_dtypes

    src0 = np.asarray(edge_index[0], dtype=np.int64)
    dst0 = np.asarray(edge_index[1], dtype=np.int64)
    n_tiles = nt_per_core * NCORES

    indeg = np.bincount(dst0, minlength=n_nodes)
    deg = (indeg + 1).astype(np.float32)          # + self loop
    dinv = (np.float32(1.0) / np.sqrt(deg)).astype(np.float32)

    tile_of, slot_of = _assign_tiles(indeg + 1, n_tiles)
    gslot = tile_of.astype(np.int64) * P + slot_of  # node -> global slot

    # --- all edges incl. explicit self-loops ---
    loops = np.arange(n_nodes, dtype=np.int64)
    src = np.concatenate([src0, loops])
    dst = np.concatenate([dst0, loops])
    n_edges = src.shape[0]

    # --- edge bucketing by dst tile (secondary: src slot, for locality) ---
    et = tile_of[dst]                              # edge -> dst tile
    order = np.lexsort((gslot[src], et))
    es, ed, et_s = src[order], dst[order], et[order]
    counts = np.bincount(et_s, minlength=n_tiles)
    C = int(math.ceil(counts.max() / P))           # chunks per tile (uniform)
    starts = np.zeros(n_tiles, np.int64)
    starts[1:] = np.cumsum(counts)[:-1]
    rank = np.arange(n_edges, dtype=np.int64) - starts[et_s]
    chunk = rank // P
    eslot = (rank % P).astype(np.int64)
    core_e = et_s // nt_per_core
    col_e = (et_s % nt_per_core) * C + chunk       # chunk column within core

    NTC = nt_per_core * C
    gidx = np.zeros((NCORES, P, NTC), np.int32)
    gidx[core_e, eslot, col_e] = gslot[es].astype(np.int32)
    # 0/1 selection matrices, fp8 (exact): S[p, col*P + dstslot] = 1
    S = np.zeros((NCORES, P, NTC * P), ml_dtypes.float8_e4m3)
    S[core_e, eslot, col_e * P + slot_of[ed]] = 1.0

    # --- per-slot activation scales: dinv^2 (mid layers), dinv (last) ---
    scm = np.zeros((NCORES, P, nt_per_core), np.float32)
    scl = np.zeros((NCORES, P, nt_per_core), np.float32)
    core_n = tile_of // nt_per_core
    lt_n = tile_of % nt_per_core
    scm[core_n, slot_of, lt_n] = dinv * dinv
    scl[core_n, slot_of, lt_n] = dinv

    return dict(gslot=gslot, dinv=dinv, C=C, gidx=gidx, S=S, scm=scm, scl=scl)


# ----------------------------------------------------------------------------
# Device program
# ----------------------------------------------------------------------------

NQUEUES = 1  # SWDGE queues; per-chunk gathers round-robin across them


def _build_program(nt_per_core, C, n_layers):
    import concourse.bass as bass
    import concourse.mybir as mybir
    import concourse.tile as tile
    from concourse import bacc
    from concourse.bass import IndirectOffsetOnAxis

    f32 = mybir.dt.float32
    f16 = mybir.dt.float16
    f8 = mybir.dt.float8e4
    SL = nt_per_core * P                 # slots per core
    NS = NCORES * SL                     # global slots
    NQ = nt_per_core // TG               # tile groups
    NTC = nt_per_core * C
    GC = TG * C                          # chunks per group

    nc = bacc.Bacc(
        "TRN2", target_bir_lowering=False, debug=False, num_devices=NCORES,
        num_swdge_queues=NQUEUES,
    )

    # Round-robin SWDGE queue assignment for indirect gathers. The HW
    # supports only 128 descriptors (one offset per partition) per
    # instruction, so gathers are per-chunk; 4 queues let 4 Q7 cores
    # generate descriptors concurrently. The queue must be set before
    # the Tile inst-callback sees the instruction, hence the wrapper.
    rr_state = {"q": 0, "on": False}
    orig_add = nc.gpsimd.add_instruction

    def add_patched(inst, *a, **kw):
        if rr_state["on"] and getattr(inst, "queue", None) == "qPoolDynamic":
            qn = rr_state["q"]
            if qn:
                inst.queue = f"qPoolDynamic{qn}"
            rr_state["q"] = (qn + 1) % NQUEUES
        return orig_add(inst, *a, **kw)

    nc.gpsimd.add_instruction = add_patched

    tok0_in = nc.dram_tensor("tok0", [P, NTC * D], f16, kind="ExternalInput")
    gidx_in = nc.dram_tensor("gidx", [P, NTC], mybir.dt.int32, kind="ExternalInput")
    S_in = nc.dram_tensor("Smat", [P, NTC * P], f8, kind="ExternalInput")
    W_in = nc.dram_tensor("Ws", [n_layers, D, D], f16, kind="ExternalInput")
    scm_in = nc.dram_tensor("scm", [P, nt_per_core], f32, kind="ExternalInput")
    scl_in = nc.dram_tensor("scl", [P, nt_per_core], f32, kind="ExternalInput")
    out_ex = nc.dram_tensor("out", [SL, D], f32, kind="ExternalOutput")

    xsh = [None] + [
        nc.dram_tensor(f"xsh{l}", [SL, D], f16) for l in range(1, n_layers)
    ]
    xfull = [None] + [
        nc.dram_tensor(f"xfull{l}", [NS, D], f16, addr_space="Shared")
        for l in range(1, n_layers)
    ]

    rg = [list(range(NCORES))]
    relu = mybir.ActivationFunctionType.Relu

    with tile.TileContext(nc) as tc:
        with (
            tc.tile_pool(name="const", bufs=1) as cp,
            tc.tile_pool(name="tokp", bufs=6) as tokp,
            tc.tile_pool(name="work", bufs=6) as work,
            tc.tile_pool(name="psA", bufs=4, space="PSUM") as psA,
            tc.tile_pool(name="psH", bufs=4, space="PSUM") as psH,
        ):
            # ---- resident constants ----
            gidx_sb = cp.tile([P, NTC], mybir.dt.int32)
            nc.sync.dma_start(gidx_sb[:], gidx_in[:])
            S_sb = cp.tile([P, NTC * P], f8)
            nc.sync.dma_start(S_sb[:], S_in[:])
            scm_sb = cp.tile([P, nt_per_core], f32)
            nc.sync.dma_start(scm_sb[:], scm_in[:])
            scl_sb = cp.tile([P, nt_per_core], f32)
            nc.sync.dma_start(scl_sb[:], scl_in[:])
            W_sb = cp.tile([P, n_layers * D], f16)
            for l in range(n_layers):
                nc.sync.dma_start(W_sb[:, l * D:(l + 1) * D], W_in[l])

            for l in range(n_layers):
                last = l == n_layers - 1
                sc_sb = scl_sb if last else scm_sb
                for q in range(NQ):
                    tok = tokp.tile([P, GC * D], f16)
                    if l == 0:
                        # layer-0 tokens are host-pregathered in edge order:
                        # one contiguous DMA per group, no gathers
                        nc.sync.dma_start(
                            tok[:], tok0_in[:, q * GC * D:(q + 1) * GC * D]
                        )
                    else:
                        # per-chunk gathers (HW limit: 128 descriptors
                        # each), spread across the SWDGE queues
                        rr_state["on"] = True
                        for k in range(GC):
                            col = q * GC + k
                            nc.gpsimd.indirect_dma_start(
                                out=tok[:, k * D:(k + 1) * D],
                                out_offset=None,
                                in_=xfull[l][:],
                                in_offset=IndirectOffsetOnAxis(
                                    ap=gidx_sb[:, col:col + 1], axis=0
                                ),
                            )
                        rr_state["on"] = False
                    psumA = psA.tile([P, TG * D], f32)
                    for j in range(TG):
                        t = q * TG + j
                        oslice = psumA[:, j * D:(j + 1) * D]
                        for c in range(C):
                            k = j * C + c
                            nc.tensor.matmul(
                                oslice,
                                tok[:, k * D:(k + 1) * D],
                                S_sb[:, (t * C + c) * P:(t * C + c + 1) * P],
                                start=(c == 0), stop=(c == C - 1),
                            )
                    # aggT (PSUM fp32) -> SBUF fp16
                    aggT = work.tile([P, TG * D], f16)
                    nc.vector.tensor_scalar_add(aggT[:], psumA[:], 0.0)
                    # h = agg @ W  (row-major out)
                    psumH = psH.tile([P, TG * D], f32)
                    for j in range(TG):
                        nc.tensor.matmul(
                            psumH[:, j * D:(j + 1) * D],
                            aggT[:, j * D:(j + 1) * D],
                            W_sb[:, l * D:(l + 1) * D],
                            start=True, stop=True,
                        )
                    # relu with per-dst-row norm scale
                    xo = work.tile([P, TG * D], f32 if last else f16)
                    for j in range(TG):
                        t = q * TG + j
                        nc.scalar.activation(
                            xo[:, j * D:(j + 1) * D],
                            psumH[:, j * D:(j + 1) * D],
                            relu, scale=sc_sb[:, t:t + 1],
                        )
                    r0 = q * TG * P
                    dst_dram = out_ex if last else xsh[l + 1]
                    nc.sync.dma_start(
                        dst_dram[r0:r0 + TG * P, :].rearrange(
                            "(g p) d -> p g d", p=P
                        ),
                        xo[:].rearrange("p (g d) -> p g d", d=D),
                    )
                if not last:
                    nc.gpsimd.collective_compute(
                        "AllGather", mybir.AluOpType.bypass, replica_groups=rg,
                        ins=[xsh[l + 1][:]], outs=[xfull[l + 1][:]],
                    )

    nc.compile()
    return nc


# ----------------------------------------------------------------------------
# Driver
# ----------------------------------------------------------------------------

def _make_in_maps(x, Ws, bs, pre, nt_per_core):
    n_layers = Ws.shape[0]
    SL = nt_per_core * P
    NS = NCORES * SL
    x = np.asarray(x, np.float32)

    assert not np.any(np.asarray(bs)), "nonzero bias not supported"

    # layer-0 features: dinv-scaled, slot-ordered
    xf0 = np.zeros((NS, D), np.float16)
    xf0[pre["gslot"]] = (x * pre["dinv"][:, None]).astype(np.float16)
    Ws16 = np.asarray(Ws, np.float16)

    in_maps = []
    for c in range(NCORES):
        # host-pregathered layer-0 tokens in edge order: [P, NTC*D]
        tok0 = xf0[pre["gidx"][c]].reshape(P, -1)
        in_maps.append({
            "tok0": tok0,
            "gidx": pre["gidx"][c],
            "Smat": pre["S"][c],
            "Ws": Ws16,
            "scm": pre["scm"][c],
            "scl": pre["scl"][c],
        })
    return in_maps


def _ensure_axon_trace_hooks():
    """This image's trn_rl_repo lacks ``antenv.axon_hooks`` (the NTFF
    profile hook shim) — synthesize it and register the ctypes hook from
    trn_agent_boot so ``run_bass_kernel_spmd(trace=True)`` can profile."""
    import types

    if "antenv.axon_hooks" not in sys.modules:
        mod = types.ModuleType("antenv.axon_hooks")
        mod._hook = None
        mod.set_axon_ntff_profile_hook = lambda h: setattr(mod, "_hook", h)
        mod.get_axon_ntff_profile_hook = lambda: mod._hook
        sys.modules["antenv.axon_hooks"] = mod
        try:
            import antenv

            antenv.axon_hooks = mod
        except Exception:
            pass
    mod = sys.modules["antenv.axon_hooks"]
    if mod.get_axon_ntff_profile_hook() is None:
        try:
            from trn_agent_boot.trn_boot import _ntff_profile_via_ctypes

            mod.set_axon_ntff_profile_hook(
                _ntff_profile_via_ctypes("/opt/axon/libaxon_pjrt.so")
            )
        except Exception as e:
            print(f"ntff hook install failed: {e}", file=sys.stderr)
    # artifact upload needs a fish bucket; keep profiles local instead.
    from concourse import bass_utils

    bass_utils.upload_artifacts = lambda tmpdir: tmpdir


def _run(x, Ws, bs, edge_index, mode="hw", trace=False, nt_per_core=104):
    n_nodes = x.shape[0]
    n_layers = Ws.shape[0]
    assert nt_per_core % TG == 0
    assert nt_per_core * P * NCORES >= n_nodes

    pre = _preprocess(edge_index, n_nodes, nt_per_core)
    C = pre["C"]

    key = (nt_per_core, C, n_layers)
    if key not in _CACHE:
        _CACHE[key] = _build_program(nt_per_core, C, n_layers)
    nc = _CACHE[key]

    in_maps = _make_in_maps(x, Ws, bs, pre, nt_per_core)

    if mode == "sim":
        from concourse.bass_interp import MultiCoreSim

        sim = MultiCoreSim(nc, num_cores=NCORES, num_workers=1, trace=False)
        cores = [sim.cores[i] for i in range(NCORES)]
        for c, cs in enumerate(cores):
            for name, arr in in_maps[c].items():
                cs.tensor(name)[:] = arr
        sim.simulate(check_with_hw=False)
        outs = [np.array(cs.tensor("out")) for cs in cores]
        res = None
    else:
        from concourse.bass_utils import run_bass_kernel_spmd

        if trace:
            _ensure_axon_trace_hooks()
        res = run_bass_kernel_spmd(
            nc, in_maps, core_ids=list(range(NCORES)), trace=trace
        )
        outs = [res.results[c]["out"] for c in range(NCORES)]

    full = np.concatenate(outs, axis=0)[pre["gslot"]]
    return np.ascontiguousarray(full, dtype=np.float32), res


def kernel(x, Ws, bs, edge_index):
    mode = os.environ.get("GCN_KERNEL_MODE", "hw")
    trace = os.environ.get("GCN_KERNEL_TRACE", "0") == "1"
    out, _ = _run(
        np.asarray(x), np.asarray(Ws), np.asarray(bs), np.asarray(edge_index),
        mode=mode, trace=trace,
    )
    return out


# ----------------------------------------------------------------------------
# Small-scale self-test (simulator)
# ----------------------------------------------------------------------------

def _ref_numpy(x, Ws, bs, edge_index):
    n = x.shape[0]
    src = np.concatenate([edge_index[0], np.arange(n)])
    dst = np.concatenate([edge_index[1], np.arange(n)])
    deg = np.bincount(dst, minlength=n).astype(np.float32)
    dinv = np.where(deg > 0, 1.0 / np.sqrt(deg), 0.0).astype(np.float32)
    norm = (dinv[src] * dinv[dst])[:, None]
    for i in range(Ws.shape[0]):
        h = x @ Ws[i]
        msg = h[src] * norm
        agg = np.zeros_like(x)
        np.add.at(agg, dst, msg)
        x = np.maximum(agg + bs[i], 0.0)
    return x


def _selftest(n_nodes=3000, n_edges=20000, n_layers=2, nt_per_core=4, seed=0):
    rng = np.random.default_rng(seed)
    x = rng.standard_normal((n_nodes, D), dtype=np.float32)
    Ws = (rng.standard_normal((n_layers, D, D)) / math.sqrt(D)).astype(np.float32)
    bs = np.zeros((n_layers, D), np.float32)
    edge_index = rng.integers(0, n_nodes, size=(2, n_edges), dtype=np.int64)

    exp = _ref_numpy(x, Ws, bs, edge_index)
    got, _ = _run(x, Ws, bs, edge_index, mode="sim", nt_per_core=nt_per_core)
    err = np.abs(got - exp)
    denom = np.abs(exp).max()
    rel = err.max() / denom
    frob = np.linalg.norm(got - exp) / np.linalg.norm(exp)
    print(f"selftest: max abs err {err.max():.3e}  rel {rel:.3e}  "
          f"frob {frob:.3e}  (denom {denom:.3f})")
    assert frob < 5e-3, "selftest FAILED"
    print("selftest PASSED")


if __name__ == "__main__":
    if "--selftest" in sys.argv:
        _selftest()
